# revision 11
# baseline (speedup 1.0000x reference)
"""Trainium2 Bass kernel for DGL HGNNConv-style hypergraph message passing.

Computation (reference):
    Xp = X @ Wlin
    Xe = segment_sum(Xp[g1_src], g1_dst, 25000) * degE * W
    Xv = segment_sum(Xe[g1_dst], g1_src, 100000) * degV

v2 strategy (8 NeuronCores, node-range sharding, gpsimd ap_gather):
  The Wlin projection commutes with both segment-sums and the row scalings,
  so all message passing runs in raw 128-feature space and Wlin is applied
  once per output tile at the end of stage 2.

  Tables are kept TRANSPOSED ([128 features, rows]) and SBUF-resident so
  gathers use nc.gpsimd.ap_gather (free-dim gather on all 8 Q7 cores,
  ~4.9 ns/idx) instead of dma_gather (SWDGE descriptor generation on 2 Q7
  cores, ~9 ns/row) — the baseline's bottleneck (GpSimd 93% busy).

  - Core m owns node rows [m*12500, (m+1)*12500) (transposed shard xT in
    SBUF) and all nnz entries whose src falls in that range.
  - Stage 1 (entries sorted by dst): per seg tile, one ap_gather pulls all
    the tile's entry columns from xT; per 128-entry chunk a PE transpose
    makes G [entry, feat] (fp16), an is_equal one-hot S [entry, seg] (fp16),
    and matmul(accT, lhsT=G, rhs=S) accumulates XeT [feat, seg] in PSUM.
    accT is scaled by degE*W (PE row-broadcast + DVE mult) and written to a
    DRAM partial, AllReduced across cores in 4 chunks overlapped with
    compute, then loaded into the SBUF XeT table [128, 25088].
  - Stage 2 (entries sorted by src): same structure per node tile, gathering
    from XeT; accT [feat, node] is scaled by degV, cast to fp16, and
    multiplied by Wlin via matmul(lhsT=wlin16, rhs=accT16) giving the
    output tile transposed [out_ch, node] — written to DRAM and
    un-transposed on host.
"""

import numpy as np

import concourse.bass as bass
import concourse.bacc as bacc
import concourse.tile as tile
import concourse.mybir as mybir
from concourse.bass_utils import run_bass_kernel_spmd
from concourse.masks import make_identity

P = 128
N_CORES = 8

N_NODES = 100000
N_HEDGES = 25000
IN_CH = 128
OUT_CH = 128
N_AR_CHUNKS = 4
TBATCH = 4  # chunks per PSUM-transpose batch


def _cdiv(a, b):
    return (a + b - 1) // b


def _wrap_idx16(idx_flat: np.ndarray) -> np.ndarray:
    """Flat index i -> partition i%16, column i//16, replicated across the
    eight 16-partition stripes (ap_gather/dma_gather wrap layout)."""
    n = idx_flat.shape[0]
    assert n % 16 == 0
    blk = idx_flat.astype(np.int16).reshape(n // 16, 16).T
    return np.tile(blk, (8, 1))


def _prep_stage(tile_key, gather_idx, local_id, n_tiles, n_cores):
    """Per-core padded gather-index / segment-id arrays with a chunk
    schedule uniform across cores (SPMD: one program)."""
    counts = np.zeros((n_cores, n_tiles), dtype=np.int64)
    slices = []
    for c in range(n_cores):
        bounds = np.searchsorted(tile_key[c], np.arange(n_tiles + 1),
                                 side="left")
        counts[c] = bounds[1:] - bounds[:-1]
        slices.append(bounds)
    chunks = np.maximum(1, _cdiv(counts.max(axis=0), P)).astype(np.int64)
    total_chunks = int(chunks.sum())
    total = total_chunks * P
    co = np.concatenate([[0], np.cumsum(chunks)])

    idx_w, ids_w = [], []
    for c in range(n_cores):
        idx_flat = np.zeros(total, dtype=np.int16)  # pad idx 0 (valid row)
        ids_flat = np.full(total, -1.0, dtype=np.float32)  # pad id -1
        bounds = slices[c]
        gi, li = gather_idx[c], local_id[c]
        for t in range(n_tiles):
            lo, hi = bounds[t], bounds[t + 1]
            cnt = hi - lo
            base = int(co[t]) * P
            idx_flat[base:base + cnt] = gi[lo:hi]
            ids_flat[base:base + cnt] = li[lo:hi]
        idx_w.append(_wrap_idx16(idx_flat))
        ids_w.append(np.ascontiguousarray(
            ids_flat.reshape(total_chunks, P).T).astype(np.float16))
    return chunks, idx_w, ids_w


def _build_program(ns_pad, seg_pad, chunks1, chunks2, n_cores):
    n_seg_tiles = seg_pad // P
    n_node_tiles = ns_pad // P
    tc1 = int(chunks1.sum())
    tc2 = int(chunks2.sum())
    co1 = np.concatenate([[0], np.cumsum(chunks1)]).astype(int)
    co2 = np.concatenate([[0], np.cumsum(chunks2)]).astype(int)
    maxch = int(max(chunks1.max(), chunks2.max()))

    nc = bacc.Bacc("TRN2", target_bir_lowering=False, debug=False,
                   num_devices=n_cores)

    xT_in = nc.dram_tensor("xT", [P, ns_pad], mybir.dt.float32,
                           kind="ExternalInput")
    wlin16_in = nc.dram_tensor("wlin16", [IN_CH, OUT_CH], mybir.dt.float16,
                               kind="ExternalInput")
    dege_in = nc.dram_tensor("dege_row", [1, seg_pad], mybir.dt.float32,
                             kind="ExternalInput")
    degv_in = nc.dram_tensor("degv_row", [1, ns_pad], mybir.dt.float32,
                             kind="ExternalInput")
    colidx_in = nc.dram_tensor("colidx16", [P, P], mybir.dt.float16,
                               kind="ExternalInput")
    ones_in = nc.dram_tensor("ones_row", [1, P], mybir.dt.float32,
                             kind="ExternalInput")
    idx1_in = nc.dram_tensor("idx1", [P, tc1 * 8], mybir.dt.int16,
                             kind="ExternalInput")
    ids1_in = nc.dram_tensor("ids1", [P, tc1], mybir.dt.float16,
                             kind="ExternalInput")
    idx2_in = nc.dram_tensor("idx2", [P, tc2 * 8], mybir.dt.int16,
                             kind="ExternalInput")
    ids2_in = nc.dram_tensor("ids2", [P, tc2], mybir.dt.float16,
                             kind="ExternalInput")
    outT = nc.dram_tensor("outT", [P, ns_pad], mybir.dt.float32,
                          kind="ExternalOutput")

    n_ar = min(N_AR_CHUNKS, n_seg_tiles)
    q_tiles = [n_seg_tiles // n_ar] * n_ar
    for i in range(n_seg_tiles % n_ar):
        q_tiles[i] += 1
    q_tile_lo = np.concatenate([[0], np.cumsum(q_tiles)]).astype(int)

    with tile.TileContext(nc) as tc:
        with (
            tc.tile_pool(name="const", bufs=1) as cpool,
            tc.tile_pool(name="idxp", bufs=3) as idxp,
            tc.tile_pool(name="idsp", bufs=3) as idsp,
            tc.tile_pool(name="gat", bufs=2) as gat,
            tc.tile_pool(name="sS", bufs=2) as sSp,
            tc.tile_pool(name="g16", bufs=3) as g16p,
            tc.tile_pool(name="ev", bufs=3) as evp,
            tc.tile_pool(name="tp", bufs=2, space="PSUM") as tpp,
            tc.tile_pool(name="acc", bufs=2, space="PSUM") as accp,
            tc.tile_pool(name="bc", bufs=2, space="PSUM") as bcp,
            tc.tile_pool(name="dram", bufs=1, space="DRAM") as dram,
        ):
            # ---- preloads ----
            xT_sb = cpool.tile([P, ns_pad], mybir.dt.float32)
            nc.sync.dma_start(xT_sb[:], xT_in[:])
            xeT_sb = cpool.tile([P, seg_pad], mybir.dt.float32)
            wlin_sb = cpool.tile([P, OUT_CH], mybir.dt.float16)
            nc.sync.dma_start(wlin_sb[:], wlin16_in[:])
            colidx_sb = cpool.tile([P, P], mybir.dt.float16)
            nc.sync.dma_start(colidx_sb[:], colidx_in[:])
            ones_sb = cpool.tile([1, P], mybir.dt.float32)
            nc.sync.dma_start(ones_sb[:], ones_in[:])
            ident = cpool.tile([P, P], mybir.dt.float32)
            make_identity(nc, ident[:])

            xT3 = xT_sb[:].rearrange("p (n d) -> p n d", d=1)
            xeT3 = xeT_sb[:].rearrange("p (n d) -> p n d", d=1)
            colidx3 = colidx_sb[:].rearrange("p (o e) -> p o e", o=1)

            xeT_part = [
                dram.tile([P, q_tiles[q] * P], mybir.dt.float32,
                          name=f"xeT_part{q}")
                for q in range(n_ar)
            ]
            xeT_red = [
                dram.tile([P, q_tiles[q] * P], mybir.dt.float32,
                          name=f"xeT_red{q}", addr_space="Shared")
                for q in range(n_ar)
            ]

            def seg_stage(t, ch, co_t, idx_in_t, ids_in_t, table3, n_elems,
                          scale_dram, scale_off, is_stage2, part_ap=None):
                n = ch * P
                idx_sb = idxp.tile([P, maxch * 8], mybir.dt.int16,
                                   tag="idx")
                nc.sync.dma_start(idx_sb[:, :ch * 8],
                                  idx_in_t[:, co_t * 8:(co_t + ch) * 8])
                ids_sb = idsp.tile([P, maxch], mybir.dt.float16, tag="ids")
                nc.sync.dma_start(ids_sb[:, :ch],
                                  ids_in_t[:, co_t:co_t + ch])
                sc_sb = idsp.tile([1, P], mybir.dt.float32, tag="sc")
                nc.sync.dma_start(sc_sb[:],
                                  scale_dram[:, scale_off:scale_off + P])
                gt = gat.tile([P, maxch * P], mybir.dt.float32, tag="gt")
                nc.gpsimd.ap_gather(
                    gt[:, :n].rearrange("p (n d) -> p n d", d=1), table3,
                    idx_sb[:, :ch * 8], P, n_elems, 1, n)
                s = sSp.tile([P, maxch * P], mybir.dt.float16, tag="s")
                nc.vector.tensor_tensor(
                    out=s[:, :n].rearrange("p (c e) -> p c e", e=P),
                    in0=ids_sb[:, :ch].to_broadcast([P, ch, P]),
                    in1=colidx3.to_broadcast([P, ch, P]),
                    op=mybir.AluOpType.is_equal,
                )
                acc = accp.tile([P, P], mybir.dt.float32, space="PSUM",
                                tag="acc")
                for b0 in range(0, ch, TBATCH):
                    bw = min(TBATCH, ch - b0)
                    tp = tpp.tile([P, TBATCH * P], mybir.dt.float32,
                                  space="PSUM", tag="tp")
                    for j in range(bw):
                        c = b0 + j
                        nc.tensor.transpose(
                            tp[:, j * P:(j + 1) * P],
                            gt[:, c * P:(c + 1) * P], ident[:])
                    gb = g16p.tile([P, TBATCH * P], mybir.dt.float16,
                                   tag="gb")
                    nc.vector.tensor_copy(gb[:, :bw * P], tp[:, :bw * P])
                    for j in range(bw):
                        c = b0 + j
                        nc.tensor.matmul(
                            acc[:], gb[:, j * P:(j + 1) * P],
                            s[:, c * P:(c + 1) * P],
                            start=(c == 0), stop=(c == ch - 1))
                # row-broadcast the scale slice into PSUM via rank-1 matmul
                bc = bcp.tile([P, P], mybir.dt.float32, space="PSUM",
                              tag="bc")
                nc.tensor.matmul(bc[:], ones_sb[:], sc_sb[:],
                                 start=True, stop=True)
                bcs = evp.tile([P, P], mybir.dt.float32, tag="bcs")
                nc.vector.tensor_copy(bcs[:], bc[:])
                if not is_stage2:
                    ev = evp.tile([P, P], mybir.dt.float32, tag="ev")
                    nc.vector.tensor_tensor(out=ev[:], in0=acc[:],
                                            in1=bcs[:],
                                            op=mybir.AluOpType.mult)
                    nc.sync.dma_start(part_ap, ev[:])
                else:
                    ev16 = evp.tile([P, P], mybir.dt.float16, tag="ev16")
                    nc.vector.tensor_tensor(out=ev16[:], in0=acc[:],
                                            in1=bcs[:],
                                            op=mybir.AluOpType.mult)
                    op = accp.tile([P, P], mybir.dt.float32, space="PSUM",
                                   tag="op")
                    nc.tensor.matmul(op[:], wlin_sb[:], ev16[:],
                                     start=True, stop=True)
                    oc = evp.tile([P, P], mybir.dt.float32, tag="oc")
                    nc.vector.tensor_copy(oc[:], op[:])
                    nc.sync.dma_start(outT[:, t * P:(t + 1) * P], oc[:])

            # ---- stage 1 + chunked AllReduce ----
            for q in range(n_ar):
                for t in range(q_tile_lo[q], q_tile_lo[q + 1]):
                    trel = t - q_tile_lo[q]
                    seg_stage(t, int(chunks1[t]), int(co1[t]), idx1_in,
                              ids1_in, xT3, ns_pad, dege_in, t * P, False,
                              part_ap=xeT_part[q][:,
                                                  trel * P:(trel + 1) * P])
                nc.gpsimd.collective_compute(
                    "AllReduce", mybir.AluOpType.add,
                    replica_groups=[list(range(n_cores))],
                    ins=[xeT_part[q].opt()],
                    outs=[xeT_red[q].opt()],
                )
                nc.sync.dma_start(
                    xeT_sb[:, q_tile_lo[q] * P:q_tile_lo[q + 1] * P],
                    xeT_red[q][:])

            # ---- stage 2 ----
            for t in range(n_node_tiles):
                seg_stage(t, int(chunks2[t]), int(co2[t]), idx2_in,
                          ids2_in, xeT3, seg_pad, degv_in, t * P, True)

    nc.compile()
    return nc


def _host_prep(X, Wlin, degE, degV, W, g1_src, g1_dst, n_cores=N_CORES):
    ns = N_NODES // n_cores
    ns_pad = _cdiv(ns, P) * P
    n_seg_tiles = _cdiv(N_HEDGES, P)
    seg_pad = n_seg_tiles * P
    n_node_tiles = ns_pad // P

    core_of = g1_src // ns

    # stage 1: per core, sorted by dst; gather from local xT by src_local
    o1 = np.lexsort((g1_dst, core_of))
    src1, dst1, c1 = g1_src[o1], g1_dst[o1], core_of[o1]
    cb1 = np.searchsorted(c1, np.arange(n_cores + 1))
    tile_key1, gidx1, lid1 = [], [], []
    for c in range(n_cores):
        lo, hi = cb1[c], cb1[c + 1]
        d = dst1[lo:hi]
        tile_key1.append(d // P)
        gidx1.append(src1[lo:hi] - c * ns)
        lid1.append((d % P).astype(np.float32))
    chunks1, idx1_w, ids1_w = _prep_stage(
        tile_key1, gidx1, lid1, n_seg_tiles, n_cores)

    # stage 2: per core, sorted by src; gather from xeT by global dst
    o2 = np.argsort(g1_src, kind="stable")
    src2, dst2 = g1_src[o2], g1_dst[o2]
    cb2 = np.searchsorted(src2, np.arange(n_cores + 1) * ns)
    tile_key2, gidx2, lid2 = [], [], []
    for c in range(n_cores):
        lo, hi = cb2[c], cb2[c + 1]
        s_local = src2[lo:hi] - c * ns
        tile_key2.append(s_local // P)
        gidx2.append(dst2[lo:hi])
        lid2.append((s_local % P).astype(np.float32))
    chunks2, idx2_w, ids2_w = _prep_stage(
        tile_key2, gidx2, lid2, n_node_tiles, n_cores)

    dege_row = np.zeros((1, seg_pad), dtype=np.float32)
    dege_row[0, :N_HEDGES] = (degE * W).reshape(-1)
    colidx16 = np.broadcast_to(
        np.arange(P, dtype=np.float16), (P, P)).copy()
    ones_row = np.ones((1, P), dtype=np.float32)
    wlin16 = np.ascontiguousarray(Wlin, dtype=np.float16)

    in_maps = []
    for c in range(n_cores):
        xT = np.zeros((P, ns_pad), dtype=np.float32)
        xT[:, :ns] = X[c * ns:(c + 1) * ns].T
        degv_row = np.zeros((1, ns_pad), dtype=np.float32)
        degv_row[0, :ns] = degV[c * ns:(c + 1) * ns].reshape(-1)
        in_maps.append({
            "xT": xT,
            "wlin16": wlin16,
            "dege_row": dege_row,
            "degv_row": degv_row,
            "colidx16": colidx16,
            "ones_row": ones_row,
            "idx1": idx1_w[c],
            "ids1": ids1_w[c],
            "idx2": idx2_w[c],
            "ids2": ids2_w[c],
        })
    return in_maps, chunks1, chunks2, ns, ns_pad, seg_pad


def run_impl(inputs: dict, trace: bool = False):
    X = np.asarray(inputs["X"], dtype=np.float32)
    Wlin = np.asarray(inputs["Wlin"], dtype=np.float32)
    degE = np.asarray(inputs["degE"], dtype=np.float32)
    degV = np.asarray(inputs["degV"], dtype=np.float32)
    W = np.asarray(inputs["W"], dtype=np.float32)
    g1_src = np.asarray(inputs["g1_src"], dtype=np.int64)
    g1_dst = np.asarray(inputs["g1_dst"], dtype=np.int64)

    in_maps, chunks1, chunks2, ns, ns_pad, seg_pad = _host_prep(
        X, Wlin, degE, degV, W, g1_src, g1_dst)
    nc = _build_program(ns_pad, seg_pad, chunks1, chunks2, N_CORES)
    res = run_bass_kernel_spmd(nc, in_maps, core_ids=list(range(N_CORES)),
                               trace=trace)
    out = np.concatenate(
        [res.results[c]["outT"][:, :ns].T for c in range(N_CORES)], axis=0)
    return np.ascontiguousarray(out), res


def kernel(**inputs) -> np.ndarray:
    out, _ = run_impl(inputs, trace=False)
    return out
